# revision 16
# baseline (speedup 1.0000x reference)
"""Causal self-attention with RoPE on 8 Trainium2 NeuronCores.

Problem (hardcoded): x (4, 2048, 2048) f32, w_attn (2048, 6144),
w_proj (2048, 2048), rope_cos/rope_sin (2048, 64), 16 heads, hd=128.

Sharding: 8 cores = 4 batches x 2 head-groups (8 heads each).  Each core
computes qkv projection for its heads, RoPE, causal attention, and a
partial output projection (its head-group's rows of w_proj).  The host
sums the two partials per batch (the "all-reduce after c_proj") and
transposes back, since the device kernel works fully transposed.

Device layout choices:
  - qT, kT stored [hd=128 partitions, T free]; S^T tiles [j_keys, q]
    come straight from matmul(lhsT=kT_j, rhs=qT_q).  Softmax exp is
    elementwise (no max subtraction needed: scores ~ N(0,1), max ~ 6);
    causality = skipping j>q blocks + masking diagonal blocks.  The PV
    matmul consumes P^T directly with v in natural [T, hd] layout as
    lhsT, producing o^T with no transposes anywhere.
  - Softmax denominators accumulate on the PE: an all-ones [128,128]
    stationary matmul sums exp(S^T) tiles over the key-partition axis
    into a PSUM tile alongside the PV accumulation; a DVE reciprocal
    off PSUM then scales o^T.  Diagonal-block masking runs on GpSimd
    so the (slow, ~3.4us) reciprocal never head-of-line-blocks the
    mask muls feeding the PE in the DVE FIFO.
  - Consecutive full-width S^T blocks pair up in one [128,1024] PSUM
    tile so exp() runs as one wide ACTIVATE (the +352-cycle fixed cost
    per instruction was ~40% of ACT exp time at width 512).
  - RoPE pairs (2i, 2i+1) are host-permuted to quadrant positions
    (32q+j, 32q+16+j) by permuting w_attn's q/k columns (dot products
    are permutation invariant), so the half-rotation partner swap is a
    single DVE stream_shuffle (within-quadrant 16-lane swap) instead
    of two SBUF round-trip DMAs.
  - Everything except PSUM accumulators and the final output runs in
    bf16: same PE rate as f32r, 2x DVE rate, half the DMA bytes, 1024
    -wide moving operands in phase A, and o^T stays SBUF-resident for
    phase C (no DRAM round trip).
"""

import sys

sys.path.insert(0, "/opt/trn_rl_repo")

import numpy as np
import ml_dtypes

import concourse.bass as bass
import concourse.mybir as mybir
import concourse.tile as tile

F32 = mybir.dt.float32
BF16 = mybir.dt.bfloat16
P = 128

# stream_shuffle mask: swap lanes 0-15 <-> 16-31 within each 32-lane quadrant
SHUF = list(range(16, 32)) + list(range(16))


# --------------------------------------------------------------------------
# This container's walrus build rejects any instruction carrying more than
# one sem wait.  Split extras onto NoOps inserted before the instruction on
# the same engine (per-engine program order makes the waits complete first).
def _split_multi_waits(nc):
    n = 0
    for fn in nc.m.functions:
        for bb in fn.blocks:
            out = []
            changed = False
            for inst in bb.instructions:
                si = inst.sync_info
                waits = list(si.on_wait or []) if si is not None else []
                if len(waits) > 1:
                    changed = True
                    n += 1
                    for w in waits[:-1]:
                        nop = mybir.InstNoOp(
                            name=nc.get_next_instruction_name(),
                            engine=inst.engine,
                            ins=[],
                            outs=[],
                            sync_info=mybir.SyncInfo(on_wait=[w], on_update=[]),
                        )
                        try:
                            nc.register_instruction(nop, overwrite=True)
                        except Exception:
                            pass
                        out.append(nop)
                    inst.sync_info = mybir.SyncInfo(
                        on_wait=[waits[-1]], on_update=list(si.on_update or [])
                    )
                out.append(inst)
            if changed:
                bb.instructions = out
    return n


def build_attention_core(T=2048, C=2048, G=8, n_half=2):
    """One core's program.  T tokens, C model dim, G heads in this core's
    group (hd=128 each).  Returns the Bass object."""
    KO = C // P          # contraction tiles over model dim
    TH = T // n_half     # tokens per phase-A pass
    NTB = TH // P        # 128-tall t blocks per half (phase A v)
    VN = min(512, G * P)  # v column chunk
    NV = (G * P) // VN
    NQ = max(T // 512, 1)  # 512-wide q chunks (phase B)
    QW = min(T, 512)
    JPQ = QW // P        # j tiles per q chunk width
    NJ = T // P          # total j tiles
    KQ = max(KO // 4, 1)  # kc per xt quarter tile
    NXQ = KO // KQ

    nc = bass.Bass()
    xt = nc.dram_tensor("xt", [n_half, P, KO, TH], BF16, kind="ExternalInput")
    wqk = nc.dram_tensor("wqk", [2 * G, P, KO, P], BF16, kind="ExternalInput")
    wv = nc.dram_tensor("wv", [NV, P, KO, VN], BF16, kind="ExternalInput")
    wp = nc.dram_tensor("wp", [P, KO, G, P], BF16, kind="ExternalInput")
    # cosd = [cos; cos], sind = [-sin; +sin] in quadrant-pair layout
    cosp = nc.dram_tensor("cosp", [P, T], BF16, kind="ExternalInput")
    sinp = nc.dram_tensor("sinp", [P, T], BF16, kind="ExternalInput")
    maskt = nc.dram_tensor("maskt", [P, P], BF16, kind="ExternalInput")
    outT = nc.dram_tensor("outT", [C, T], F32, kind="ExternalOutput")

    scale = 1.0 / np.sqrt(128.0)
    EXP = mybir.ActivationFunctionType.Exp
    CPY = mybir.ActivationFunctionType.Copy

    with tile.TileContext(nc) as tc:
        with (
            tc.tile_pool(name="dram", bufs=1, space="DRAM") as dram,
            tc.tile_pool(name="const", bufs=1) as cpool,
        ):
            qkd = dram.tile([2 * G, P, T], BF16)

            # gpsimd (SWDGE) queue order: 2 x-quarters first, then rope
            # constants, then the H1 x tiles, then w_proj (phase C only);
            # sync/scalar queues start on the other x / w tiles immediately
            cos_s = cpool.tile([P, T], BF16)
            sin_s = cpool.tile([P, T], BF16)
            mask_s = cpool.tile([P, P], BF16)
            ones_bf = cpool.tile([P, P], BF16)
            nc.vector.memset(ones_bf[:], 1.0)
            wp_s = cpool.tile([P, KO, G, P], BF16)

            def rope_head(pool_set, psqk, m, t0):
                qkbf_pool, rtmp_pool, roped_pool = pool_set
                qk_bf = qkbf_pool.tile([P, TH], BF16, tag="qkbf")
                nc.scalar.activation(qk_bf[:], psqk[:], CPY)
                sw = rtmp_pool.tile([P, TH], BF16, tag="rtmp")
                nc.vector.stream_shuffle(sw[:], qk_bf[:], SHUF)
                rop = roped_pool.tile([P, TH], BF16, tag="roped")
                nc.vector.tensor_mul(
                    rop[:], qk_bf[:], cos_s[:, t0 : t0 + TH]
                )
                nc.vector.tensor_mul(sw[:], sw[:], sin_s[:, t0 : t0 + TH])
                nc.vector.tensor_add(rop[:], rop[:], sw[:])
                nc.gpsimd.dma_start(qkd[m, :, t0 : t0 + TH], rop[:])

            with tc.tile_pool(name="vall", bufs=1) as va_pool:
                # v stays resident in SBUF through phases A and B:
                # v_all[ti, to, hh*128+d] = v[to*128+ti, head hh, d]
                v_all = va_pool.tile([P, NJ, G * P], BF16, tag="vall")
                oacc_tiles = []

                # ------- Phase A: qkT + RoPE, v (v first in half 1) -------
                with (
                    tc.tile_pool(name="xt", bufs=2 * NXQ) as xt_pool,
                    tc.tile_pool(name="wqk", bufs=3) as wqk_pool,
                    tc.tile_pool(name="wv", bufs=2) as wv_pool,
                    tc.tile_pool(name="qkbf", bufs=2) as qkbf_pool,
                    tc.tile_pool(name="roped", bufs=2) as roped_pool,
                    tc.tile_pool(name="ropetmp", bufs=2) as rtmp_pool,
                    tc.tile_pool(name="psA", bufs=2, space="PSUM") as psA,
                    tc.tile_pool(name="psV", bufs=2, space="PSUM") as psV,
                ):
                    pool_set = (qkbf_pool, rtmp_pool, roped_pool)
                    xtq = {}

                    def load_x(H, engs):
                        for qq in range(NXQ):
                            xq = xt_pool.tile([P, KQ, TH], BF16, tag="xtq",
                                              name=f"xtq{H}_{qq}")
                            engs[qq % len(engs)].dma_start(
                                xq[:], xt[H, :, qq * KQ : (qq + 1) * KQ, :]
                            )
                            xtq[(H, qq)] = xq

                    def load_x_split(H):
                        # first-needed quarters spread over the idle queues;
                        # quarter 1 rides scalar behind the first two w tiles
                        for qq, eng in ((0, nc.sync), (2, nc.sync),
                                        (3, nc.sync), (1, nc.scalar)):
                            xq = xt_pool.tile([P, KQ, TH], BF16, tag="xtq",
                                              name=f"xtq{H}_{qq}")
                            eng.dma_start(
                                xq[:], xt[H, :, qq * KQ : (qq + 1) * KQ, :]
                            )
                            xtq[(H, qq)] = xq

                    # q head m and k head m+G interleave so phase B head m
                    # unblocks right after both its projections finish
                    M_ORDER = [m for pair in zip(range(G), range(G, 2 * G))
                               for m in pair]

                    def load_w(H, m, w_tiles):
                        w_s = wqk_pool.tile([P, KO, P], BF16, tag="wqk",
                                            name=f"wqk{H}_{m}")
                        nc.scalar.dma_start(w_s[:], wqk[m])
                        w_tiles[m] = w_s

                    def qk_heads(H, w_tiles, hooks=None):
                        t0 = H * TH
                        # prefetch stays 2 heads ahead; never queue a DMA
                        # that waits on a pool slot (it would head-of-line
                        # block the scalar engine FIFO and with it every
                        # rope copy behind it)
                        for idx, m in enumerate(M_ORDER):
                            if hooks and idx in hooks:
                                hooks[idx]()
                            if idx + 2 < len(M_ORDER):
                                load_w(H, M_ORDER[idx + 2], w_tiles)
                            w_s = w_tiles[m]
                            psqk = psA.tile([P, TH], F32, tag="pqk")
                            for kc in range(KO):
                                for i in range(TH // 512):
                                    nc.tensor.matmul(
                                        psqk[:, i * 512 : (i + 1) * 512],
                                        w_s[:, kc, :],
                                        xtq[(H, kc // KQ)][
                                            :, kc % KQ,
                                            i * 512 : (i + 1) * 512,
                                        ],
                                        start=(kc == 0),
                                        stop=(kc == KO - 1),
                                        skip_group_check=True,
                                    )
                            rope_head(pool_set, psqk, m, t0)

                    wv_tiles = {}

                    def load_wv():
                        for n2 in range(NV):
                            wv_s = wv_pool.tile([P, KO, VN], BF16, tag="wv",
                                                name=f"wv{n2}")
                            nc.sync.dma_start(wv_s[:], wv[n2])
                            wv_tiles[n2] = wv_s

                    def v_blocks(H):
                        for n2 in range(NV):
                            wv_s = wv_tiles[n2]
                            for tb in range(NTB):
                                psv = psV.tile([P, VN], F32, tag="pv")
                                for kc in range(KO):
                                    nc.tensor.matmul(
                                        psv[:],
                                        xtq[(H, kc // KQ)][
                                            :, kc % KQ, tb * P : (tb + 1) * P
                                        ],
                                        wv_s[:, kc, :],
                                        start=(kc == 0),
                                        stop=(kc == KO - 1),
                                    )
                                nc.vector.tensor_copy(
                                    v_all[
                                        :, H * NTB + tb,
                                        n2 * VN : (n2 + 1) * VN,
                                    ],
                                    psv[:],
                                )

                    # half 0: qk first (first matmul needs just one x
                    # quarter + one 0.5MB w tile); half 1: v first so
                    # v_all completes before phase B needs its tail.
                    # Queues: sync = x(H0) + wv + phase-B q/k reads;
                    # scalar = w_qk; gpsimd = consts/wp + x(H1) + qkd
                    # writes (keeps each HWDGE FIFO free of cross-phase
                    # head-of-line blocking).
                    w_tiles0, w_tiles1 = {}, {}
                    load_w(0, M_ORDER[0], w_tiles0)
                    load_w(0, M_ORDER[1], w_tiles0)
                    load_x_split(0)
                    nc.gpsimd.dma_start(cos_s[:], cosp[:])
                    nc.gpsimd.dma_start(sin_s[:], sinp[:])
                    nc.gpsimd.dma_start(mask_s[:], maskt[:])
                    load_wv()
                    # bulk transfers not needed until much later are emitted
                    # a few heads in, so they don't steal HBM bandwidth from
                    # the critical first x/w tiles
                    qk_heads(0, w_tiles0, hooks={
                        3: lambda: load_x(1, [nc.gpsimd]),
                        8: lambda: nc.gpsimd.dma_start(wp_s[:], wp[:]),
                    })
                    v_blocks(0)
                    load_w(1, M_ORDER[0], w_tiles1)
                    load_w(1, M_ORDER[1], w_tiles1)
                    v_blocks(1)
                    qk_heads(1, w_tiles1)

                # ------------- Phase B: attention per head -------------
                with (
                    tc.tile_pool(name="qh", bufs=3) as q_pool,
                    tc.tile_pool(name="kh", bufs=3) as k_pool,
                    tc.tile_pool(name="pt", bufs=6) as pt_pool,
                    tc.tile_pool(name="rinv", bufs=2) as rinv_pool,
                    tc.tile_pool(name="oacc", bufs=G) as oacc_pool,
                    tc.tile_pool(name="psS", bufs=2, space="PSUM") as psS,
                    tc.tile_pool(name="psO", bufs=2, space="PSUM") as psO,
                    tc.tile_pool(name="psR", bufs=2, space="PSUM") as psR,
                ):
                    # Pack J blocks into [P, 2*QW] PSUM tiles: two
                    # consecutive full-width (co=0) blocks share a tile and
                    # one exp(); diagonal blocks (co>0) go solo.  PV/sums
                    # trail by >= KEEP finalized members so ACT exp latency
                    # stays hidden, and the pipeline runs across Q and head
                    # boundaries so the PE never drains at them.
                    ready = []
                    KEEP = 4

                    def drain(upto):
                        while len(ready) > upto:
                            (Jp, cop, pTp, sl, pso, psr, jmax,
                             oT, Q, h) = ready.pop(0)
                            nc.tensor.matmul(
                                pso[:, cop:],
                                v_all[:, Jp, h * P : (h + 1) * P],
                                pTp[:, sl],
                                start=(Jp == 0),
                                stop=(Jp == jmax),
                                skip_group_check=True,
                            )
                            nc.tensor.matmul(
                                psr[:, cop:],
                                ones_bf[:],
                                pTp[:, sl],
                                start=(Jp == 0),
                                stop=(Jp == jmax),
                                skip_group_check=True,
                            )
                            if Jp == jmax:
                                rinv = rinv_pool.tile([P, QW], F32,
                                                      tag="rinv", name="rinv")
                                nc.vector.reciprocal(
                                    rinv[:, : QW // 2], psr[:, : QW // 2]
                                )
                                nc.vector.reciprocal(
                                    rinv[:, QW // 2 :], psr[:, QW // 2 :]
                                )
                                nc.vector.tensor_mul(
                                    oT[:, Q * QW : (Q + 1) * QW],
                                    pso[:], rinv[:],
                                )

                    qk_tiles = {}

                    def load_qk(h):
                        qT = q_pool.tile([P, T], BF16, tag="q",
                                         name=f"qT{h}")
                        nc.sync.dma_start(qT[:], qkd[h])
                        kT = k_pool.tile([P, T], BF16, tag="k",
                                         name=f"kT{h}")
                        nc.sync.dma_start(kT[:], qkd[G + h])
                        qk_tiles[h] = (qT, kT)

                    load_qk(0)
                    load_qk(1)
                    for h in range(G):
                        if h + 2 < G:
                            load_qk(h + 2)
                        qT, kT = qk_tiles[h]
                        oT = oacc_pool.tile([P, T], BF16, tag="oacc",
                                            name=f"oacc{h}")
                        oacc_tiles.append(oT)
                        for Q in range(NQ):
                            jmax = JPQ * (Q + 1) - 1  # inclusive
                            pso = psO.tile([P, QW], F32, tag="pso")
                            psr = psR.tile([P, QW], F32, tag="psr")

                            def finalize(tile_ps, members):
                                c0 = members[0][1] + members[0][2] * QW
                                c1 = members[-1][2] * QW + QW
                                pT = pt_pool.tile([P, 2 * QW], BF16, tag="pt")
                                nc.scalar.activation(
                                    pT[:, c0:c1], tile_ps[:, c0:c1], EXP,
                                    scale=scale,
                                )
                                for J, co, s in members:
                                    if J - JPQ * Q >= 0:  # diagonal: mask
                                        nc.gpsimd.tensor_mul(
                                            pT[:, s * QW + co :
                                               s * QW + co + P],
                                            pT[:, s * QW + co :
                                               s * QW + co + P],
                                            mask_s[:],
                                        )
                                    ready.append(
                                        (J, co, pT,
                                         slice(s * QW + co, (s + 1) * QW),
                                         pso, psr, jmax, oT, Q, h)
                                    )
                                drain(KEEP)

                            cur = None  # (psS tile, members)
                            for J in range(jmax + 1):
                                k_d = J - JPQ * Q  # diag idx if >= 0
                                co = max(k_d, 0) * P
                                if cur is None:
                                    cur = (psS.tile([P, 2 * QW], F32,
                                                    tag="pss", name="pss"),
                                           [])
                                tile_ps, members = cur
                                slot = len(members)
                                nc.tensor.matmul(
                                    tile_ps[:, slot * QW + co :
                                            (slot + 1) * QW],
                                    kT[:, J * P : (J + 1) * P],
                                    qT[:, Q * QW + co : (Q + 1) * QW],
                                    start=True,
                                    stop=True,
                                    skip_group_check=True,
                                )
                                members.append((J, co, slot))
                                nxt_co = (max(J + 1 - JPQ * Q, 0)) * P
                                if (len(members) == 2 or co > 0
                                        or nxt_co > 0 or J == jmax):
                                    finalize(tile_ps, members)
                                    cur = None
                    drain(0)

                # ------------- Phase C: output projection -------------
                with (
                    tc.tile_pool(name="csb", bufs=4) as csb_pool,
                    tc.tile_pool(name="psC", bufs=4, space="PSUM") as psC,
                ):
                    for t in range(NQ):
                        for m in range(KO):
                            psc = psC.tile([P, QW], F32, tag="psc")
                            for h in range(G):
                                nc.tensor.matmul(
                                    psc[:],
                                    wp_s[:, m, h, :],
                                    oacc_tiles[h][:, t * QW : (t + 1) * QW],
                                    start=(h == 0),
                                    stop=(h == G - 1),
                                )
                            csb = csb_pool.tile([P, QW], F32, tag="csb")
                            nc.scalar.activation(csb[:], psc[:], CPY)
                            eng = nc.sync if m % 2 == 0 else nc.scalar
                            eng.dma_start(
                                outT[m * P : (m + 1) * P,
                                     t * QW : (t + 1) * QW],
                                csb[:],
                            )

    _split_multi_waits(nc)
    return nc


# --------------------------------------------------------------------------
def _prep_core_inputs(xb, w_attn, w_proj, rope_cos, rope_sin, g, G=8, n_half=2):
    """Host-side shard prep for one core: batch slice xb (T, C), group g."""
    T, C = xb.shape
    KO = C // P
    TH = T // n_half
    VN = min(512, G * P)
    NV = (G * P) // VN
    gc = g * G * P  # column offset of this group within one qkv section
    bf16 = ml_dtypes.bfloat16

    # x^T arranged [half, ki, ko, t]
    xtT = np.ascontiguousarray(xb.T)  # (C, T)
    xt = np.ascontiguousarray(
        xtT.reshape(KO, P, n_half, TH).transpose(2, 1, 0, 3)
    )

    # q,k columns for this group, RoPE pair-permuted so pair j of hd sits at
    # quadrant positions (32*(j//16) + j%16, +16): the half-rotation partner
    # is then a within-quadrant 16-lane swap (stream_shuffle-able)
    perm = np.empty(P, dtype=np.int64)
    j = np.arange(64)
    perm[32 * (j // 16) + (j % 16)] = 2 * j
    perm[32 * (j // 16) + 16 + (j % 16)] = 2 * j + 1
    wq = w_attn[:, gc : gc + G * P].reshape(C, G, P)[:, :, perm]
    wk = w_attn[:, C + gc : C + gc + G * P].reshape(C, G, P)[:, :, perm]
    wqk_cols = np.concatenate(
        [wq.reshape(C, G * P), wk.reshape(C, G * P)], axis=1
    )  # (C, 2*G*128)
    wqk = np.ascontiguousarray(
        wqk_cols.reshape(KO, P, 2 * G, P).transpose(2, 1, 0, 3)
    )

    wv_cols = w_attn[:, 2 * C + gc : 2 * C + gc + G * P]  # (C, G*128)
    wv = np.ascontiguousarray(
        wv_cols.reshape(KO, P, NV, VN).transpose(2, 1, 0, 3)
    )

    wp_rows = w_proj[gc : gc + G * P, :]  # (G*128, C)
    wp = np.ascontiguousarray(
        wp_rows.reshape(G, P, KO, P).transpose(1, 2, 0, 3)
    )  # (P, KO, G, P)

    # cos/sin in the same quadrant-pair layout; sind = [-sin; +sin]
    cT = rope_cos[:T].T  # (64, T)
    sT = rope_sin[:T].T
    cospT = np.empty((P, T), dtype=np.float32)
    sinpT = np.empty((P, T), dtype=np.float32)
    pos1 = 32 * (j // 16) + (j % 16)
    pos2 = pos1 + 16
    cospT[pos1] = cT
    cospT[pos2] = cT
    sinpT[pos1] = -sT
    sinpT[pos2] = sT
    mask = np.triu(np.ones((P, P), dtype=np.float32))

    return {
        "xt": xt.astype(bf16),
        "wqk": wqk.astype(bf16),
        "wv": wv.astype(bf16),
        "wp": wp.astype(bf16),
        "cosp": cospT.astype(bf16),
        "sinp": sinpT.astype(bf16),
        "maskt": mask.astype(bf16),
    }


_NC_CACHE = {}
TRACE = False
LAST_RESULTS = None


def kernel(x, w_attn, w_proj, rope_cos, rope_sin):
    from concourse.bass_utils import run_bass_kernel_spmd

    x = np.asarray(x, dtype=np.float32)
    w_attn = np.asarray(w_attn, dtype=np.float32)
    w_proj = np.asarray(w_proj, dtype=np.float32)
    rope_cos = np.asarray(rope_cos, dtype=np.float32)
    rope_sin = np.asarray(rope_sin, dtype=np.float32)

    B, T, C = x.shape
    G = 8  # heads per group (16 heads / 2 groups)

    key = (T, C, G)
    if key not in _NC_CACHE:
        _NC_CACHE[key] = build_attention_core(T=T, C=C, G=G, n_half=2)
    nc = _NC_CACHE[key]

    in_maps = []
    for core in range(8):
        b, g = core // 2, core % 2
        in_maps.append(
            _prep_core_inputs(x[b], w_attn, w_proj, rope_cos, rope_sin, g, G=G)
        )

    res = run_bass_kernel_spmd(nc, in_maps, list(range(8)), trace=TRACE)
    global LAST_RESULTS
    LAST_RESULTS = res

    y = np.empty((B, T, C), dtype=np.float32)
    for b in range(B):
        acc = res.results[2 * b]["outT"] + res.results[2 * b + 1]["outT"]
        y[b] = acc.T
    return y


# revision 17
# speedup vs baseline: 1.0155x; 1.0155x over previous
"""Causal self-attention with RoPE on 8 Trainium2 NeuronCores.

Problem (hardcoded): x (4, 2048, 2048) f32, w_attn (2048, 6144),
w_proj (2048, 2048), rope_cos/rope_sin (2048, 64), 16 heads, hd=128.

Sharding: 8 cores = 4 batches x 2 head-groups (8 heads each).  Each core
computes qkv projection for its heads, RoPE, causal attention, and a
partial output projection (its head-group's rows of w_proj).  The host
sums the two partials per batch (the "all-reduce after c_proj") and
transposes back, since the device kernel works fully transposed.

Device layout choices:
  - qT, kT stored [hd=128 partitions, T free]; S^T tiles [j_keys, q]
    come straight from matmul(lhsT=kT_j, rhs=qT_q).  Softmax exp is
    elementwise (no max subtraction needed: scores ~ N(0,1), max ~ 6);
    causality = skipping j>q blocks + masking diagonal blocks.  The PV
    matmul consumes P^T directly with v in natural [T, hd] layout as
    lhsT, producing o^T with no transposes anywhere.
  - Softmax denominators accumulate on the PE: an all-ones [128,128]
    stationary matmul sums exp(S^T) tiles over the key-partition axis
    into a PSUM tile alongside the PV accumulation; a DVE reciprocal
    off PSUM then scales o^T.  Diagonal-block masking runs on GpSimd
    so the (slow, ~3.4us) reciprocal never head-of-line-blocks the
    mask muls feeding the PE in the DVE FIFO.
  - Consecutive full-width S^T blocks pair up in one [128,1024] PSUM
    tile so exp() runs as one wide ACTIVATE (the +352-cycle fixed cost
    per instruction was ~40% of ACT exp time at width 512).
  - RoPE pairs (2i, 2i+1) are host-permuted to quadrant positions
    (32q+j, 32q+16+j) by permuting w_attn's q/k columns (dot products
    are permutation invariant), so the half-rotation partner swap is a
    single DVE stream_shuffle (within-quadrant 16-lane swap) instead
    of two SBUF round-trip DMAs.
  - Everything except PSUM accumulators and the final output runs in
    bf16: same PE rate as f32r, 2x DVE rate, half the DMA bytes, 1024
    -wide moving operands in phase A, and o^T stays SBUF-resident for
    phase C (no DRAM round trip).
"""

import sys

sys.path.insert(0, "/opt/trn_rl_repo")

import numpy as np
import ml_dtypes

import concourse.bass as bass
import concourse.mybir as mybir
import concourse.tile as tile

F32 = mybir.dt.float32
BF16 = mybir.dt.bfloat16
P = 128

# stream_shuffle mask: swap lanes 0-15 <-> 16-31 within each 32-lane quadrant
SHUF = list(range(16, 32)) + list(range(16))


# --------------------------------------------------------------------------
# This container's walrus build rejects any instruction carrying more than
# one sem wait.  Split extras onto NoOps inserted before the instruction on
# the same engine (per-engine program order makes the waits complete first).
def _split_multi_waits(nc):
    n = 0
    for fn in nc.m.functions:
        for bb in fn.blocks:
            out = []
            changed = False
            for inst in bb.instructions:
                si = inst.sync_info
                waits = list(si.on_wait or []) if si is not None else []
                if len(waits) > 1:
                    changed = True
                    n += 1
                    for w in waits[:-1]:
                        nop = mybir.InstNoOp(
                            name=nc.get_next_instruction_name(),
                            engine=inst.engine,
                            ins=[],
                            outs=[],
                            sync_info=mybir.SyncInfo(on_wait=[w], on_update=[]),
                        )
                        try:
                            nc.register_instruction(nop, overwrite=True)
                        except Exception:
                            pass
                        out.append(nop)
                    inst.sync_info = mybir.SyncInfo(
                        on_wait=[waits[-1]], on_update=list(si.on_update or [])
                    )
                out.append(inst)
            if changed:
                bb.instructions = out
    return n


def build_attention_core(T=2048, C=2048, G=8, n_half=2):
    """One core's program.  T tokens, C model dim, G heads in this core's
    group (hd=128 each).  Returns the Bass object."""
    KO = C // P          # contraction tiles over model dim
    TH = T // n_half     # tokens per phase-A pass
    NTB = TH // P        # 128-tall t blocks per half (phase A v)
    VN = min(512, G * P)  # v column chunk
    NV = (G * P) // VN
    NQ = max(T // 512, 1)  # 512-wide q chunks (phase B)
    QW = min(T, 512)
    JPQ = QW // P        # j tiles per q chunk width
    NJ = T // P          # total j tiles
    KQ = max(KO // 4, 1)  # kc per xt quarter tile
    NXQ = KO // KQ

    nc = bass.Bass()
    xt = nc.dram_tensor("xt", [n_half, P, KO, TH], BF16, kind="ExternalInput")
    wqk = nc.dram_tensor("wqk", [2 * G, P, KO, P], BF16, kind="ExternalInput")
    wv = nc.dram_tensor("wv", [NV, P, KO, VN], BF16, kind="ExternalInput")
    wp = nc.dram_tensor("wp", [P, KO, G, P], BF16, kind="ExternalInput")
    # cosd = [cos; cos], sind = [-sin; +sin] in quadrant-pair layout
    cosp = nc.dram_tensor("cosp", [P, T], BF16, kind="ExternalInput")
    sinp = nc.dram_tensor("sinp", [P, T], BF16, kind="ExternalInput")
    maskt = nc.dram_tensor("maskt", [P, P], BF16, kind="ExternalInput")
    outT = nc.dram_tensor("outT", [C, T], F32, kind="ExternalOutput")

    scale = 1.0 / np.sqrt(128.0)
    EXP = mybir.ActivationFunctionType.Exp
    CPY = mybir.ActivationFunctionType.Copy

    with tile.TileContext(nc) as tc:
        with (
            tc.tile_pool(name="dram", bufs=1, space="DRAM") as dram,
            tc.tile_pool(name="const", bufs=1) as cpool,
        ):
            qkd = [dram.tile([P, T], BF16, name=f"qkd{m}")
                   for m in range(2 * G)]

            # gpsimd (SWDGE) queue order: 2 x-quarters first, then rope
            # constants, then the H1 x tiles, then w_proj (phase C only);
            # sync/scalar queues start on the other x / w tiles immediately
            cos_s = cpool.tile([P, T], BF16)
            sin_s = cpool.tile([P, T], BF16)
            mask_s = cpool.tile([P, P], BF16)
            ones_bf = cpool.tile([P, P], BF16)
            nc.vector.memset(ones_bf[:], 1.0)
            wp_s = cpool.tile([P, KO, G, P], BF16)

            def rope_head(pool_set, psqk, m, t0):
                qkbf_pool, rtmp_pool, roped_pool = pool_set
                qk_bf = qkbf_pool.tile([P, TH], BF16, tag="qkbf")
                nc.scalar.activation(qk_bf[:], psqk[:], CPY)
                sw = rtmp_pool.tile([P, TH], BF16, tag="rtmp")
                nc.vector.stream_shuffle(sw[:], qk_bf[:], SHUF)
                rop = roped_pool.tile([P, TH], BF16, tag="roped")
                nc.vector.tensor_mul(
                    rop[:], qk_bf[:], cos_s[:, t0 : t0 + TH]
                )
                nc.vector.tensor_mul(sw[:], sw[:], sin_s[:, t0 : t0 + TH])
                nc.vector.tensor_add(rop[:], rop[:], sw[:])
                nc.gpsimd.dma_start(qkd[m][:, t0 : t0 + TH], rop[:])

            with tc.tile_pool(name="vall", bufs=1) as va_pool:
                # v stays resident in SBUF through phases A and B:
                # v_all[ti, to, hh*128+d] = v[to*128+ti, head hh, d]
                v_all = va_pool.tile([P, NJ, G * P], BF16, tag="vall")
                oacc_tiles = []

                # ------- Phase A: qkT + RoPE, v (v first in half 1) -------
                with (
                    tc.tile_pool(name="xt", bufs=2 * NXQ) as xt_pool,
                    tc.tile_pool(name="wqk", bufs=3) as wqk_pool,
                    tc.tile_pool(name="wv", bufs=2) as wv_pool,
                    tc.tile_pool(name="qkbf", bufs=2) as qkbf_pool,
                    tc.tile_pool(name="roped", bufs=2) as roped_pool,
                    tc.tile_pool(name="ropetmp", bufs=2) as rtmp_pool,
                    tc.tile_pool(name="psA", bufs=2, space="PSUM") as psA,
                    tc.tile_pool(name="psV", bufs=2, space="PSUM") as psV,
                ):
                    pool_set = (qkbf_pool, rtmp_pool, roped_pool)
                    xtq = {}

                    def load_x(H, engs):
                        for qq in range(NXQ):
                            xq = xt_pool.tile([P, KQ, TH], BF16, tag="xtq",
                                              name=f"xtq{H}_{qq}")
                            engs[qq % len(engs)].dma_start(
                                xq[:], xt[H, :, qq * KQ : (qq + 1) * KQ, :]
                            )
                            xtq[(H, qq)] = xq

                    def load_x_split(H):
                        # first-needed quarters alternate sync/gpsimd (the
                        # scalar HWDGE ring measures ~3x slower; keep it on
                        # the small w tiles only)
                        for qq, eng in ((0, nc.sync), (1, nc.gpsimd),
                                        (2, nc.sync), (3, nc.gpsimd)):
                            xq = xt_pool.tile([P, KQ, TH], BF16, tag="xtq",
                                              name=f"xtq{H}_{qq}")
                            eng.dma_start(
                                xq[:], xt[H, :, qq * KQ : (qq + 1) * KQ, :]
                            )
                            xtq[(H, qq)] = xq

                    # q head m and k head m+G interleave so phase B head m
                    # unblocks right after both its projections finish
                    M_ORDER = [m for pair in zip(range(G), range(G, 2 * G))
                               for m in pair]

                    def load_w(H, m, w_tiles):
                        w_s = wqk_pool.tile([P, KO, P], BF16, tag="wqk",
                                            name=f"wqk{H}_{m}")
                        nc.scalar.dma_start(w_s[:], wqk[m])
                        w_tiles[m] = w_s

                    def qk_heads(H, w_tiles, hooks=None):
                        t0 = H * TH
                        # prefetch stays 2 heads ahead; never queue a DMA
                        # that waits on a pool slot (it would head-of-line
                        # block the scalar engine FIFO and with it every
                        # rope copy behind it)
                        for idx, m in enumerate(M_ORDER):
                            if hooks and idx in hooks:
                                hooks[idx]()
                            if idx + 2 < len(M_ORDER):
                                load_w(H, M_ORDER[idx + 2], w_tiles)
                            w_s = w_tiles[m]
                            psqk = psA.tile([P, TH], F32, tag="pqk")
                            for kc in range(KO):
                                for i in range(TH // 512):
                                    nc.tensor.matmul(
                                        psqk[:, i * 512 : (i + 1) * 512],
                                        w_s[:, kc, :],
                                        xtq[(H, kc // KQ)][
                                            :, kc % KQ,
                                            i * 512 : (i + 1) * 512,
                                        ],
                                        start=(kc == 0),
                                        stop=(kc == KO - 1),
                                        skip_group_check=True,
                                    )
                            rope_head(pool_set, psqk, m, t0)

                    wv_tiles = {}

                    def load_wv():
                        for n2 in range(NV):
                            wv_s = wv_pool.tile([P, KO, VN], BF16, tag="wv",
                                                name=f"wv{n2}")
                            nc.sync.dma_start(wv_s[:], wv[n2])
                            wv_tiles[n2] = wv_s

                    def v_blocks(H):
                        for n2 in range(NV):
                            wv_s = wv_tiles[n2]
                            for tb in range(NTB):
                                psv = psV.tile([P, VN], F32, tag="pv")
                                for kc in range(KO):
                                    nc.tensor.matmul(
                                        psv[:],
                                        xtq[(H, kc // KQ)][
                                            :, kc % KQ, tb * P : (tb + 1) * P
                                        ],
                                        wv_s[:, kc, :],
                                        start=(kc == 0),
                                        stop=(kc == KO - 1),
                                    )
                                nc.vector.tensor_copy(
                                    v_all[
                                        :, H * NTB + tb,
                                        n2 * VN : (n2 + 1) * VN,
                                    ],
                                    psv[:],
                                )

                    # half 0: qk first (first matmul needs just one x
                    # quarter + one 0.5MB w tile); half 1: v first so
                    # v_all completes before phase B needs its tail.
                    # Queues: sync = x(H0) + wv + phase-B q/k reads;
                    # scalar = w_qk; gpsimd = consts/wp + x(H1) + qkd
                    # writes (keeps each HWDGE FIFO free of cross-phase
                    # head-of-line blocking).
                    w_tiles0, w_tiles1 = {}, {}
                    load_w(0, M_ORDER[0], w_tiles0)
                    load_w(0, M_ORDER[1], w_tiles0)
                    load_x_split(0)
                    nc.gpsimd.dma_start(cos_s[:], cosp[:])
                    nc.gpsimd.dma_start(sin_s[:], sinp[:])
                    nc.gpsimd.dma_start(mask_s[:], maskt[:])
                    load_wv()
                    # bulk transfers not needed until much later are emitted
                    # a few heads in, so they don't steal HBM bandwidth from
                    # the critical first x/w tiles
                    qk_heads(0, w_tiles0, hooks={
                        3: lambda: load_x(1, [nc.gpsimd]),
                        8: lambda: nc.gpsimd.dma_start(wp_s[:], wp[:]),
                    })
                    v_blocks(0)
                    load_w(1, M_ORDER[0], w_tiles1)
                    load_w(1, M_ORDER[1], w_tiles1)
                    v_blocks(1)
                    qk_heads(1, w_tiles1)

                # ------------- Phase B: attention per head -------------
                with (
                    tc.tile_pool(name="qh", bufs=3) as q_pool,
                    tc.tile_pool(name="kh", bufs=3) as k_pool,
                    tc.tile_pool(name="pt", bufs=6) as pt_pool,
                    tc.tile_pool(name="rinv", bufs=2) as rinv_pool,
                    tc.tile_pool(name="oacc", bufs=G) as oacc_pool,
                    tc.tile_pool(name="psS", bufs=2, space="PSUM") as psS,
                    tc.tile_pool(name="psO", bufs=2, space="PSUM") as psO,
                    tc.tile_pool(name="psR", bufs=2, space="PSUM") as psR,
                ):
                    # Pack J blocks into [P, 2*QW] PSUM tiles: two
                    # consecutive full-width (co=0) blocks share a tile and
                    # one exp(); diagonal blocks (co>0) go solo.  PV/sums
                    # trail by >= KEEP finalized members so ACT exp latency
                    # stays hidden, and the pipeline runs across Q and head
                    # boundaries so the PE never drains at them.
                    ready = []
                    KEEP = 4

                    def drain(upto):
                        while len(ready) > upto:
                            (Jp, cop, pTp, sl, pso, psr, jmax,
                             oT, Q, h) = ready.pop(0)
                            nc.tensor.matmul(
                                pso[:, cop:],
                                v_all[:, Jp, h * P : (h + 1) * P],
                                pTp[:, sl],
                                start=(Jp == 0),
                                stop=(Jp == jmax),
                                skip_group_check=True,
                            )
                            nc.tensor.matmul(
                                psr[:, cop:],
                                ones_bf[:],
                                pTp[:, sl],
                                start=(Jp == 0),
                                stop=(Jp == jmax),
                                skip_group_check=True,
                            )
                            if Jp == jmax:
                                rinv = rinv_pool.tile([P, QW], F32,
                                                      tag="rinv", name="rinv")
                                nc.vector.reciprocal(
                                    rinv[:, : QW // 2], psr[:, : QW // 2]
                                )
                                nc.vector.reciprocal(
                                    rinv[:, QW // 2 :], psr[:, QW // 2 :]
                                )
                                nc.vector.tensor_mul(
                                    oT[:, Q * QW : (Q + 1) * QW],
                                    pso[:], rinv[:],
                                )

                    qk_tiles = {}

                    def load_qk(h):
                        qT = q_pool.tile([P, T], BF16, tag="q",
                                         name=f"qT{h}")
                        nc.sync.dma_start(qT[:], qkd[h][:])
                        kT = k_pool.tile([P, T], BF16, tag="k",
                                         name=f"kT{h}")
                        nc.sync.dma_start(kT[:], qkd[G + h][:])
                        qk_tiles[h] = (qT, kT)

                    load_qk(0)
                    load_qk(1)
                    for h in range(G):
                        if h + 2 < G:
                            load_qk(h + 2)
                        qT, kT = qk_tiles[h]
                        oT = oacc_pool.tile([P, T], BF16, tag="oacc",
                                            name=f"oacc{h}")
                        oacc_tiles.append(oT)
                        for Q in range(NQ):
                            jmax = JPQ * (Q + 1) - 1  # inclusive
                            pso = psO.tile([P, QW], F32, tag="pso")
                            psr = psR.tile([P, QW], F32, tag="psr")

                            def finalize(tile_ps, members):
                                c0 = members[0][1] + members[0][2] * QW
                                c1 = members[-1][2] * QW + QW
                                pT = pt_pool.tile([P, 2 * QW], BF16, tag="pt")
                                nc.scalar.activation(
                                    pT[:, c0:c1], tile_ps[:, c0:c1], EXP,
                                    scale=scale,
                                )
                                for J, co, s in members:
                                    if J - JPQ * Q >= 0:  # diagonal: mask
                                        nc.gpsimd.tensor_mul(
                                            pT[:, s * QW + co :
                                               s * QW + co + P],
                                            pT[:, s * QW + co :
                                               s * QW + co + P],
                                            mask_s[:],
                                        )
                                    ready.append(
                                        (J, co, pT,
                                         slice(s * QW + co, (s + 1) * QW),
                                         pso, psr, jmax, oT, Q, h)
                                    )
                                drain(KEEP)

                            cur = None  # (psS tile, members)
                            for J in range(jmax + 1):
                                k_d = J - JPQ * Q  # diag idx if >= 0
                                co = max(k_d, 0) * P
                                if cur is None:
                                    cur = (psS.tile([P, 2 * QW], F32,
                                                    tag="pss", name="pss"),
                                           [])
                                tile_ps, members = cur
                                slot = len(members)
                                nc.tensor.matmul(
                                    tile_ps[:, slot * QW + co :
                                            (slot + 1) * QW],
                                    kT[:, J * P : (J + 1) * P],
                                    qT[:, Q * QW + co : (Q + 1) * QW],
                                    start=True,
                                    stop=True,
                                    skip_group_check=True,
                                )
                                members.append((J, co, slot))
                                nxt_co = (max(J + 1 - JPQ * Q, 0)) * P
                                if (len(members) == 2 or co > 0
                                        or nxt_co > 0 or J == jmax):
                                    finalize(tile_ps, members)
                                    cur = None
                    drain(0)

                # ------------- Phase C: output projection -------------
                with (
                    tc.tile_pool(name="csb", bufs=4) as csb_pool,
                    tc.tile_pool(name="psC", bufs=4, space="PSUM") as psC,
                ):
                    for t in range(NQ):
                        for m in range(KO):
                            psc = psC.tile([P, QW], F32, tag="psc")
                            for h in range(G):
                                nc.tensor.matmul(
                                    psc[:],
                                    wp_s[:, m, h, :],
                                    oacc_tiles[h][:, t * QW : (t + 1) * QW],
                                    start=(h == 0),
                                    stop=(h == G - 1),
                                )
                            csb = csb_pool.tile([P, QW], F32, tag="csb")
                            nc.scalar.activation(csb[:], psc[:], CPY)
                            eng = (nc.sync, nc.scalar, nc.gpsimd)[m % 3]
                            eng.dma_start(
                                outT[m * P : (m + 1) * P,
                                     t * QW : (t + 1) * QW],
                                csb[:],
                            )

    _split_multi_waits(nc)
    return nc


# --------------------------------------------------------------------------
def _prep_core_inputs(xb, w_attn, w_proj, rope_cos, rope_sin, g, G=8, n_half=2):
    """Host-side shard prep for one core: batch slice xb (T, C), group g."""
    T, C = xb.shape
    KO = C // P
    TH = T // n_half
    VN = min(512, G * P)
    NV = (G * P) // VN
    gc = g * G * P  # column offset of this group within one qkv section
    bf16 = ml_dtypes.bfloat16

    # x^T arranged [half, ki, ko, t]
    xtT = np.ascontiguousarray(xb.T)  # (C, T)
    xt = np.ascontiguousarray(
        xtT.reshape(KO, P, n_half, TH).transpose(2, 1, 0, 3)
    )

    # q,k columns for this group, RoPE pair-permuted so pair j of hd sits at
    # quadrant positions (32*(j//16) + j%16, +16): the half-rotation partner
    # is then a within-quadrant 16-lane swap (stream_shuffle-able)
    perm = np.empty(P, dtype=np.int64)
    j = np.arange(64)
    perm[32 * (j // 16) + (j % 16)] = 2 * j
    perm[32 * (j // 16) + 16 + (j % 16)] = 2 * j + 1
    wq = w_attn[:, gc : gc + G * P].reshape(C, G, P)[:, :, perm]
    wk = w_attn[:, C + gc : C + gc + G * P].reshape(C, G, P)[:, :, perm]
    wqk_cols = np.concatenate(
        [wq.reshape(C, G * P), wk.reshape(C, G * P)], axis=1
    )  # (C, 2*G*128)
    wqk = np.ascontiguousarray(
        wqk_cols.reshape(KO, P, 2 * G, P).transpose(2, 1, 0, 3)
    )

    wv_cols = w_attn[:, 2 * C + gc : 2 * C + gc + G * P]  # (C, G*128)
    wv = np.ascontiguousarray(
        wv_cols.reshape(KO, P, NV, VN).transpose(2, 1, 0, 3)
    )

    wp_rows = w_proj[gc : gc + G * P, :]  # (G*128, C)
    wp = np.ascontiguousarray(
        wp_rows.reshape(G, P, KO, P).transpose(1, 2, 0, 3)
    )  # (P, KO, G, P)

    # cos/sin in the same quadrant-pair layout; sind = [-sin; +sin]
    cT = rope_cos[:T].T  # (64, T)
    sT = rope_sin[:T].T
    cospT = np.empty((P, T), dtype=np.float32)
    sinpT = np.empty((P, T), dtype=np.float32)
    pos1 = 32 * (j // 16) + (j % 16)
    pos2 = pos1 + 16
    cospT[pos1] = cT
    cospT[pos2] = cT
    sinpT[pos1] = -sT
    sinpT[pos2] = sT
    mask = np.triu(np.ones((P, P), dtype=np.float32))

    return {
        "xt": xt.astype(bf16),
        "wqk": wqk.astype(bf16),
        "wv": wv.astype(bf16),
        "wp": wp.astype(bf16),
        "cosp": cospT.astype(bf16),
        "sinp": sinpT.astype(bf16),
        "maskt": mask.astype(bf16),
    }


_NC_CACHE = {}
TRACE = False
LAST_RESULTS = None


def kernel(x, w_attn, w_proj, rope_cos, rope_sin):
    from concourse.bass_utils import run_bass_kernel_spmd

    x = np.asarray(x, dtype=np.float32)
    w_attn = np.asarray(w_attn, dtype=np.float32)
    w_proj = np.asarray(w_proj, dtype=np.float32)
    rope_cos = np.asarray(rope_cos, dtype=np.float32)
    rope_sin = np.asarray(rope_sin, dtype=np.float32)

    B, T, C = x.shape
    G = 8  # heads per group (16 heads / 2 groups)

    key = (T, C, G)
    if key not in _NC_CACHE:
        _NC_CACHE[key] = build_attention_core(T=T, C=C, G=G, n_half=2)
    nc = _NC_CACHE[key]

    in_maps = []
    for core in range(8):
        b, g = core // 2, core % 2
        in_maps.append(
            _prep_core_inputs(x[b], w_attn, w_proj, rope_cos, rope_sin, g, G=G)
        )

    res = run_bass_kernel_spmd(nc, in_maps, list(range(8)), trace=TRACE)
    global LAST_RESULTS
    LAST_RESULTS = res

    y = np.empty((B, T, C), dtype=np.float32)
    for b in range(B):
        acc = res.results[2 * b]["outT"] + res.results[2 * b + 1]["outT"]
        y[b] = acc.T
    return y


# revision 18
# speedup vs baseline: 1.0198x; 1.0042x over previous
"""Causal self-attention with RoPE on 8 Trainium2 NeuronCores.

Problem (hardcoded): x (4, 2048, 2048) f32, w_attn (2048, 6144),
w_proj (2048, 2048), rope_cos/rope_sin (2048, 64), 16 heads, hd=128.

Sharding: 8 cores = 4 batches x 2 head-groups (8 heads each).  Each core
computes qkv projection for its heads, RoPE, causal attention, and a
partial output projection (its head-group's rows of w_proj).  The host
sums the two partials per batch (the "all-reduce after c_proj") and
transposes back, since the device kernel works fully transposed.

Device layout choices:
  - qT, kT stored [hd=128 partitions, T free]; S^T tiles [j_keys, q]
    come straight from matmul(lhsT=kT_j, rhs=qT_q).  Softmax exp is
    elementwise (no max subtraction needed: scores ~ N(0,1), max ~ 6);
    causality = skipping j>q blocks + masking diagonal blocks.  The PV
    matmul consumes P^T directly with v in natural [T, hd] layout as
    lhsT, producing o^T with no transposes anywhere.
  - Softmax denominators accumulate on the PE: an all-ones [128,128]
    stationary matmul sums exp(S^T) tiles over the key-partition axis
    into a PSUM tile alongside the PV accumulation; a DVE reciprocal
    off PSUM then scales o^T.  Diagonal-block masking runs on GpSimd
    so the (slow, ~3.4us) reciprocal never head-of-line-blocks the
    mask muls feeding the PE in the DVE FIFO.
  - Consecutive full-width S^T blocks pair up in one [128,1024] PSUM
    tile so exp() runs as one wide ACTIVATE (the +352-cycle fixed cost
    per instruction was ~40% of ACT exp time at width 512).
  - RoPE pairs (2i, 2i+1) are host-permuted to quadrant positions
    (32q+j, 32q+16+j) by permuting w_attn's q/k columns (dot products
    are permutation invariant), so the half-rotation partner swap is a
    single DVE stream_shuffle (within-quadrant 16-lane swap) instead
    of two SBUF round-trip DMAs.
  - Everything except PSUM accumulators and the final output runs in
    bf16: same PE rate as f32r, 2x DVE rate, half the DMA bytes, 1024
    -wide moving operands in phase A, and o^T stays SBUF-resident for
    phase C (no DRAM round trip).
"""

import sys

sys.path.insert(0, "/opt/trn_rl_repo")

import numpy as np
import ml_dtypes

import concourse.bass as bass
import concourse.mybir as mybir
import concourse.tile as tile

F32 = mybir.dt.float32
BF16 = mybir.dt.bfloat16
P = 128

# stream_shuffle mask: swap lanes 0-15 <-> 16-31 within each 32-lane quadrant
SHUF = list(range(16, 32)) + list(range(16))


# --------------------------------------------------------------------------
# This container's walrus build rejects any instruction carrying more than
# one sem wait.  Split extras onto NoOps inserted before the instruction on
# the same engine (per-engine program order makes the waits complete first).
def _split_multi_waits(nc):
    n = 0
    for fn in nc.m.functions:
        for bb in fn.blocks:
            out = []
            changed = False
            for inst in bb.instructions:
                si = inst.sync_info
                waits = list(si.on_wait or []) if si is not None else []
                if len(waits) > 1:
                    changed = True
                    n += 1
                    for w in waits[:-1]:
                        nop = mybir.InstNoOp(
                            name=nc.get_next_instruction_name(),
                            engine=inst.engine,
                            ins=[],
                            outs=[],
                            sync_info=mybir.SyncInfo(on_wait=[w], on_update=[]),
                        )
                        try:
                            nc.register_instruction(nop, overwrite=True)
                        except Exception:
                            pass
                        out.append(nop)
                    inst.sync_info = mybir.SyncInfo(
                        on_wait=[waits[-1]], on_update=list(si.on_update or [])
                    )
                out.append(inst)
            if changed:
                bb.instructions = out
    return n


def build_attention_core(T=2048, C=2048, G=8, n_half=2):
    """One core's program.  T tokens, C model dim, G heads in this core's
    group (hd=128 each).  Returns the Bass object."""
    KO = C // P          # contraction tiles over model dim
    TH = T // n_half     # tokens per phase-A pass
    NTB = TH // P        # 128-tall t blocks per half (phase A v)
    VN = min(512, G * P)  # v column chunk
    NV = (G * P) // VN
    NQ = max(T // 512, 1)  # 512-wide q chunks (phase B)
    QW = min(T, 512)
    JPQ = QW // P        # j tiles per q chunk width
    NJ = T // P          # total j tiles
    KQ = max(KO // 4, 1)  # kc per xt quarter tile
    NXQ = KO // KQ

    nc = bass.Bass()
    xt = nc.dram_tensor("xt", [n_half, P, KO, TH], BF16, kind="ExternalInput")
    wqk = nc.dram_tensor("wqk", [2 * G, P, KO, P], BF16, kind="ExternalInput")
    wv = nc.dram_tensor("wv", [NV, P, KO, VN], BF16, kind="ExternalInput")
    wp = nc.dram_tensor("wp", [P, KO, G, P], BF16, kind="ExternalInput")
    # cosd = [cos; cos], sind = [-sin; +sin] in quadrant-pair layout
    cosp = nc.dram_tensor("cosp", [P, T], BF16, kind="ExternalInput")
    sinp = nc.dram_tensor("sinp", [P, T], BF16, kind="ExternalInput")
    maskt = nc.dram_tensor("maskt", [P, P], BF16, kind="ExternalInput")
    outT = nc.dram_tensor("outT", [C, T], F32, kind="ExternalOutput")

    scale = 1.0 / np.sqrt(128.0)
    EXP = mybir.ActivationFunctionType.Exp
    CPY = mybir.ActivationFunctionType.Copy

    with tile.TileContext(nc) as tc:
        with (
            tc.tile_pool(name="dram", bufs=1, space="DRAM") as dram,
            tc.tile_pool(name="const", bufs=1) as cpool,
        ):
            qkd = [dram.tile([P, T], BF16, name=f"qkd{m}")
                   for m in range(2 * G)]

            # gpsimd (SWDGE) queue order: 2 x-quarters first, then rope
            # constants, then the H1 x tiles, then w_proj (phase C only);
            # sync/scalar queues start on the other x / w tiles immediately
            cos_s = cpool.tile([P, T], BF16)
            sin_s = cpool.tile([P, T], BF16)
            mask_s = cpool.tile([P, P], BF16)
            ones_bf = cpool.tile([P, P], BF16)
            nc.vector.memset(ones_bf[:], 1.0)
            wp_s = cpool.tile([P, KO, G, P], BF16)

            def rope_head(pool_set, psqk, m, t0):
                qkbf_pool, rtmp_pool, roped_pool = pool_set
                qk_bf = qkbf_pool.tile([P, TH], BF16, tag="qkbf")
                nc.scalar.activation(qk_bf[:], psqk[:], CPY)
                sw = rtmp_pool.tile([P, TH], BF16, tag="rtmp")
                nc.vector.stream_shuffle(sw[:], qk_bf[:], SHUF)
                rop = roped_pool.tile([P, TH], BF16, tag="roped")
                nc.vector.tensor_mul(
                    rop[:], qk_bf[:], cos_s[:, t0 : t0 + TH]
                )
                nc.vector.tensor_mul(sw[:], sw[:], sin_s[:, t0 : t0 + TH])
                nc.vector.tensor_add(rop[:], rop[:], sw[:])
                nc.gpsimd.dma_start(qkd[m][:, t0 : t0 + TH], rop[:])

            with tc.tile_pool(name="vall", bufs=1) as va_pool:
                # v stays resident in SBUF through phases A and B:
                # v_all[ti, to, hh*128+d] = v[to*128+ti, head hh, d]
                v_all = va_pool.tile([P, NJ, G * P], BF16, tag="vall")
                oacc_tiles = []

                # ------- Phase A: qkT + RoPE, v (v first in half 1) -------
                with (
                    tc.tile_pool(name="xt", bufs=2 * NXQ) as xt_pool,
                    tc.tile_pool(name="wqk", bufs=3) as wqk_pool,
                    tc.tile_pool(name="wv", bufs=2) as wv_pool,
                    tc.tile_pool(name="qkbf", bufs=2) as qkbf_pool,
                    tc.tile_pool(name="roped", bufs=2) as roped_pool,
                    tc.tile_pool(name="ropetmp", bufs=2) as rtmp_pool,
                    tc.tile_pool(name="psA", bufs=2, space="PSUM") as psA,
                    tc.tile_pool(name="psV", bufs=2, space="PSUM") as psV,
                ):
                    pool_set = (qkbf_pool, rtmp_pool, roped_pool)
                    xtq = {}

                    def load_x(H, engs):
                        for qq in range(NXQ):
                            xq = xt_pool.tile([P, KQ, TH], BF16, tag="xtq",
                                              name=f"xtq{H}_{qq}")
                            engs[qq % len(engs)].dma_start(
                                xq[:], xt[H, :, qq * KQ : (qq + 1) * KQ, :]
                            )
                            xtq[(H, qq)] = xq

                    def load_x_split(H):
                        # first-needed quarters alternate sync/gpsimd (the
                        # scalar HWDGE ring measures ~3x slower; keep it on
                        # the small w tiles only)
                        for qq, eng in ((0, nc.sync), (1, nc.gpsimd),
                                        (2, nc.sync), (3, nc.gpsimd)):
                            xq = xt_pool.tile([P, KQ, TH], BF16, tag="xtq",
                                              name=f"xtq{H}_{qq}")
                            eng.dma_start(
                                xq[:], xt[H, :, qq * KQ : (qq + 1) * KQ, :]
                            )
                            xtq[(H, qq)] = xq

                    # q head m and k head m+G interleave so phase B head m
                    # unblocks right after both its projections finish
                    M_ORDER = [m for pair in zip(range(G), range(G, 2 * G))
                               for m in pair]

                    def load_w(H, m, w_tiles, idx=0):
                        w_s = wqk_pool.tile([P, KO, P], BF16, tag="wqk",
                                            name=f"wqk{H}_{m}")
                        # the scalar HWDGE ring moves ~45GB/s vs sync's
                        # ~115GB/s: alternate so neither starves the 6.8us
                        # per-head consumption rate (sync is busy with x
                        # for the first ~20us, so the earliest live on
                        # scalar)
                        eng = nc.scalar if (idx < 2 or idx % 2 == 0) else nc.sync
                        eng.dma_start(w_s[:], wqk[m])
                        w_tiles[m] = w_s

                    def qk_heads(H, w_tiles, hooks=None):
                        t0 = H * TH
                        # prefetch stays 2 heads ahead; never queue a DMA
                        # that waits on a pool slot (it would head-of-line
                        # block the scalar engine FIFO and with it every
                        # rope copy behind it)
                        for idx, m in enumerate(M_ORDER):
                            if hooks and idx in hooks:
                                hooks[idx]()
                            if idx + 2 < len(M_ORDER):
                                load_w(H, M_ORDER[idx + 2], w_tiles,
                                       idx=idx + 2)
                            w_s = w_tiles[m]
                            psqk = psA.tile([P, TH], F32, tag="pqk")
                            for kc in range(KO):
                                for i in range(TH // 512):
                                    nc.tensor.matmul(
                                        psqk[:, i * 512 : (i + 1) * 512],
                                        w_s[:, kc, :],
                                        xtq[(H, kc // KQ)][
                                            :, kc % KQ,
                                            i * 512 : (i + 1) * 512,
                                        ],
                                        start=(kc == 0),
                                        stop=(kc == KO - 1),
                                        skip_group_check=True,
                                    )
                            rope_head(pool_set, psqk, m, t0)

                    wv_tiles = {}

                    def load_wv():
                        for n2 in range(NV):
                            wv_s = wv_pool.tile([P, KO, VN], BF16, tag="wv",
                                                name=f"wv{n2}")
                            nc.sync.dma_start(wv_s[:], wv[n2])
                            wv_tiles[n2] = wv_s

                    def v_blocks(H):
                        for n2 in range(NV):
                            wv_s = wv_tiles[n2]
                            for tb in range(NTB):
                                psv = psV.tile([P, VN], F32, tag="pv")
                                for kc in range(KO):
                                    nc.tensor.matmul(
                                        psv[:],
                                        xtq[(H, kc // KQ)][
                                            :, kc % KQ, tb * P : (tb + 1) * P
                                        ],
                                        wv_s[:, kc, :],
                                        start=(kc == 0),
                                        stop=(kc == KO - 1),
                                    )
                                nc.vector.tensor_copy(
                                    v_all[
                                        :, H * NTB + tb,
                                        n2 * VN : (n2 + 1) * VN,
                                    ],
                                    psv[:],
                                )

                    # half 0: qk first (first matmul needs just one x
                    # quarter + one 0.5MB w tile); half 1: v first so
                    # v_all completes before phase B needs its tail.
                    # Queues: sync = x(H0) + wv + phase-B q/k reads;
                    # scalar = w_qk; gpsimd = consts/wp + x(H1) + qkd
                    # writes (keeps each HWDGE FIFO free of cross-phase
                    # head-of-line blocking).
                    w_tiles0, w_tiles1 = {}, {}
                    load_w(0, M_ORDER[0], w_tiles0, idx=0)
                    load_w(0, M_ORDER[1], w_tiles0, idx=1)
                    load_x_split(0)
                    nc.gpsimd.dma_start(cos_s[:], cosp[:])
                    nc.gpsimd.dma_start(sin_s[:], sinp[:])
                    nc.gpsimd.dma_start(mask_s[:], maskt[:])
                    load_wv()
                    # bulk transfers not needed until much later are emitted
                    # a few heads in, so they don't steal HBM bandwidth from
                    # the critical first x/w tiles
                    qk_heads(0, w_tiles0, hooks={
                        3: lambda: load_x(1, [nc.gpsimd]),
                        8: lambda: nc.gpsimd.dma_start(wp_s[:], wp[:]),
                    })
                    v_blocks(0)
                    load_w(1, M_ORDER[0], w_tiles1, idx=0)
                    load_w(1, M_ORDER[1], w_tiles1, idx=1)
                    v_blocks(1)
                    qk_heads(1, w_tiles1)

                # ------------- Phase B: attention per head -------------
                with (
                    tc.tile_pool(name="qh", bufs=3) as q_pool,
                    tc.tile_pool(name="kh", bufs=3) as k_pool,
                    tc.tile_pool(name="pt", bufs=6) as pt_pool,
                    tc.tile_pool(name="rinv", bufs=2) as rinv_pool,
                    tc.tile_pool(name="oacc", bufs=G) as oacc_pool,
                    tc.tile_pool(name="psS", bufs=2, space="PSUM") as psS,
                    tc.tile_pool(name="psO", bufs=2, space="PSUM") as psO,
                    tc.tile_pool(name="psR", bufs=2, space="PSUM") as psR,
                ):
                    # Pack J blocks into [P, 2*QW] PSUM tiles: two
                    # consecutive full-width (co=0) blocks share a tile and
                    # one exp(); diagonal blocks (co>0) go solo.  PV/sums
                    # trail by >= KEEP finalized members so ACT exp latency
                    # stays hidden, and the pipeline runs across Q and head
                    # boundaries so the PE never drains at them.
                    ready = []
                    KEEP = 3

                    def drain(upto):
                        while len(ready) > upto:
                            (Jp, cop, pTp, sl, pso, psr, jmax,
                             oT, Q, h) = ready.pop(0)
                            nc.tensor.matmul(
                                pso[:, cop:],
                                v_all[:, Jp, h * P : (h + 1) * P],
                                pTp[:, sl],
                                start=(Jp == 0),
                                stop=(Jp == jmax),
                                skip_group_check=True,
                            )
                            nc.tensor.matmul(
                                psr[:, cop:],
                                ones_bf[:],
                                pTp[:, sl],
                                start=(Jp == 0),
                                stop=(Jp == jmax),
                                skip_group_check=True,
                            )
                            if Jp == jmax:
                                rinv = rinv_pool.tile([P, QW], F32,
                                                      tag="rinv", name="rinv")
                                nc.vector.reciprocal(
                                    rinv[:, : QW // 2], psr[:, : QW // 2]
                                )
                                nc.vector.reciprocal(
                                    rinv[:, QW // 2 :], psr[:, QW // 2 :]
                                )
                                nc.vector.tensor_mul(
                                    oT[:, Q * QW : (Q + 1) * QW],
                                    pso[:], rinv[:],
                                )

                    qk_tiles = {}

                    def load_qk(h):
                        qT = q_pool.tile([P, T], BF16, tag="q",
                                         name=f"qT{h}")
                        nc.sync.dma_start(qT[:], qkd[h][:])
                        kT = k_pool.tile([P, T], BF16, tag="k",
                                         name=f"kT{h}")
                        nc.sync.dma_start(kT[:], qkd[G + h][:])
                        qk_tiles[h] = (qT, kT)

                    load_qk(0)
                    load_qk(1)
                    for h in range(G):
                        if h + 2 < G:
                            load_qk(h + 2)
                        qT, kT = qk_tiles[h]
                        oT = oacc_pool.tile([P, T], BF16, tag="oacc",
                                            name=f"oacc{h}")
                        oacc_tiles.append(oT)
                        for Q in range(NQ):
                            jmax = JPQ * (Q + 1) - 1  # inclusive
                            pso = psO.tile([P, QW], F32, tag="pso")
                            psr = psR.tile([P, QW], F32, tag="psr")

                            def finalize(tile_ps, members):
                                c0 = members[0][1] + members[0][2] * QW
                                c1 = members[-1][2] * QW + QW
                                pT = pt_pool.tile([P, 2 * QW], BF16, tag="pt")
                                nc.scalar.activation(
                                    pT[:, c0:c1], tile_ps[:, c0:c1], EXP,
                                    scale=scale,
                                )
                                for J, co, s in members:
                                    if J - JPQ * Q >= 0:  # diagonal: mask
                                        nc.gpsimd.tensor_mul(
                                            pT[:, s * QW + co :
                                               s * QW + co + P],
                                            pT[:, s * QW + co :
                                               s * QW + co + P],
                                            mask_s[:],
                                        )
                                    ready.append(
                                        (J, co, pT,
                                         slice(s * QW + co, (s + 1) * QW),
                                         pso, psr, jmax, oT, Q, h)
                                    )
                                drain(KEEP)

                            cur = None  # (psS tile, members)
                            for J in range(jmax + 1):
                                k_d = J - JPQ * Q  # diag idx if >= 0
                                co = max(k_d, 0) * P
                                if cur is None:
                                    cur = (psS.tile([P, 2 * QW], F32,
                                                    tag="pss", name="pss"),
                                           [])
                                tile_ps, members = cur
                                slot = len(members)
                                nc.tensor.matmul(
                                    tile_ps[:, slot * QW + co :
                                            (slot + 1) * QW],
                                    kT[:, J * P : (J + 1) * P],
                                    qT[:, Q * QW + co : (Q + 1) * QW],
                                    start=True,
                                    stop=True,
                                    skip_group_check=True,
                                )
                                members.append((J, co, slot))
                                nxt_co = (max(J + 1 - JPQ * Q, 0)) * P
                                if (len(members) == 2 or co > 0
                                        or nxt_co > 0 or J == jmax):
                                    finalize(tile_ps, members)
                                    cur = None
                    drain(0)

                # ------------- Phase C: output projection -------------
                with (
                    tc.tile_pool(name="csb", bufs=4) as csb_pool,
                    tc.tile_pool(name="psC", bufs=4, space="PSUM") as psC,
                ):
                    for t in range(NQ):
                        for m in range(KO):
                            psc = psC.tile([P, QW], F32, tag="psc")
                            for h in range(G):
                                nc.tensor.matmul(
                                    psc[:],
                                    wp_s[:, m, h, :],
                                    oacc_tiles[h][:, t * QW : (t + 1) * QW],
                                    start=(h == 0),
                                    stop=(h == G - 1),
                                )
                            csb = csb_pool.tile([P, QW], F32, tag="csb")
                            nc.scalar.activation(csb[:], psc[:], CPY)
                            eng = (nc.sync, nc.scalar, nc.gpsimd)[m % 3]
                            eng.dma_start(
                                outT[m * P : (m + 1) * P,
                                     t * QW : (t + 1) * QW],
                                csb[:],
                            )

    _split_multi_waits(nc)
    return nc


# --------------------------------------------------------------------------
def _prep_core_inputs(xb, w_attn, w_proj, rope_cos, rope_sin, g, G=8, n_half=2):
    """Host-side shard prep for one core: batch slice xb (T, C), group g."""
    T, C = xb.shape
    KO = C // P
    TH = T // n_half
    VN = min(512, G * P)
    NV = (G * P) // VN
    gc = g * G * P  # column offset of this group within one qkv section
    bf16 = ml_dtypes.bfloat16

    # x^T arranged [half, ki, ko, t]
    xtT = np.ascontiguousarray(xb.T)  # (C, T)
    xt = np.ascontiguousarray(
        xtT.reshape(KO, P, n_half, TH).transpose(2, 1, 0, 3)
    )

    # q,k columns for this group, RoPE pair-permuted so pair j of hd sits at
    # quadrant positions (32*(j//16) + j%16, +16): the half-rotation partner
    # is then a within-quadrant 16-lane swap (stream_shuffle-able)
    perm = np.empty(P, dtype=np.int64)
    j = np.arange(64)
    perm[32 * (j // 16) + (j % 16)] = 2 * j
    perm[32 * (j // 16) + 16 + (j % 16)] = 2 * j + 1
    wq = w_attn[:, gc : gc + G * P].reshape(C, G, P)[:, :, perm]
    wk = w_attn[:, C + gc : C + gc + G * P].reshape(C, G, P)[:, :, perm]
    wqk_cols = np.concatenate(
        [wq.reshape(C, G * P), wk.reshape(C, G * P)], axis=1
    )  # (C, 2*G*128)
    wqk = np.ascontiguousarray(
        wqk_cols.reshape(KO, P, 2 * G, P).transpose(2, 1, 0, 3)
    )

    wv_cols = w_attn[:, 2 * C + gc : 2 * C + gc + G * P]  # (C, G*128)
    wv = np.ascontiguousarray(
        wv_cols.reshape(KO, P, NV, VN).transpose(2, 1, 0, 3)
    )

    wp_rows = w_proj[gc : gc + G * P, :]  # (G*128, C)
    wp = np.ascontiguousarray(
        wp_rows.reshape(G, P, KO, P).transpose(1, 2, 0, 3)
    )  # (P, KO, G, P)

    # cos/sin in the same quadrant-pair layout; sind = [-sin; +sin]
    cT = rope_cos[:T].T  # (64, T)
    sT = rope_sin[:T].T
    cospT = np.empty((P, T), dtype=np.float32)
    sinpT = np.empty((P, T), dtype=np.float32)
    pos1 = 32 * (j // 16) + (j % 16)
    pos2 = pos1 + 16
    cospT[pos1] = cT
    cospT[pos2] = cT
    sinpT[pos1] = -sT
    sinpT[pos2] = sT
    mask = np.triu(np.ones((P, P), dtype=np.float32))

    return {
        "xt": xt.astype(bf16),
        "wqk": wqk.astype(bf16),
        "wv": wv.astype(bf16),
        "wp": wp.astype(bf16),
        "cosp": cospT.astype(bf16),
        "sinp": sinpT.astype(bf16),
        "maskt": mask.astype(bf16),
    }


_NC_CACHE = {}
TRACE = False
LAST_RESULTS = None


def kernel(x, w_attn, w_proj, rope_cos, rope_sin):
    from concourse.bass_utils import run_bass_kernel_spmd

    x = np.asarray(x, dtype=np.float32)
    w_attn = np.asarray(w_attn, dtype=np.float32)
    w_proj = np.asarray(w_proj, dtype=np.float32)
    rope_cos = np.asarray(rope_cos, dtype=np.float32)
    rope_sin = np.asarray(rope_sin, dtype=np.float32)

    B, T, C = x.shape
    G = 8  # heads per group (16 heads / 2 groups)

    key = (T, C, G)
    if key not in _NC_CACHE:
        _NC_CACHE[key] = build_attention_core(T=T, C=C, G=G, n_half=2)
    nc = _NC_CACHE[key]

    in_maps = []
    for core in range(8):
        b, g = core // 2, core % 2
        in_maps.append(
            _prep_core_inputs(x[b], w_attn, w_proj, rope_cos, rope_sin, g, G=G)
        )

    res = run_bass_kernel_spmd(nc, in_maps, list(range(8)), trace=TRACE)
    global LAST_RESULTS
    LAST_RESULTS = res

    y = np.empty((B, T, C), dtype=np.float32)
    for b in range(B):
        acc = res.results[2 * b]["outT"] + res.results[2 * b + 1]["outT"]
        y[b] = acc.T
    return y
